# revision 9
# baseline (speedup 1.0000x reference)
"""Trainium2 Bass kernel for nn_BottleneckS4D (8-core SPMD).

Strategy (self-contained, hardcoded):
  The reference is  u = x_flat @ Wb.T + bb  (256 x 150528 @ 150528 x 1280,
  770MB weight) followed by an S4D block whose output is only consumed at
  the LAST timestep (readout takes y[:, -1, :]), so the FFT convolution
  collapses to a per-channel dot product over time with the reversed S4D
  kernel, and everything downstream is tiny.

  Sharding: split the CONTRACTION dim D_IN=150528 across the 8 cores
  (18816 each). Each core streams its 96MB weight slice + 19MB x slice
  once (total HBM traffic = one pass over the data, the minimum), and
  computes a partial u^T (1280, 256) in PSUM with fp32r matmuls (full
  bf16-rate, ~1e-4 relative error). The S4D conv is linear in u, so each
  core reduces its partial u to a partial y_last (1280, 4) and a single
  tiny AllReduce (20KB) produces the exact y_last everywhere. GELU, the
  GLU 1x1 conv (sharded 160 channels/core + AllGather), and the readout
  MLP run on-device on every core; core 0's output is returned.

  Perf details: weights/x are host-repacked to partition-major layout
  (wTp[p, k, :] = wT[k*128+p, :]) so each DMA chunk moves CH k-tiles with
  one large contiguous descriptor per partition; wt streams on the sync
  HWDGE queue while xt + small tensors use the scalar HWDGE queue; a
  PE warmup burst (zeros matmuls into a scratch PSUM bank) lifts the HAM
  clock gate before the real matmuls arrive; a dummy 128B AllReduce early
  in the kernel absorbs the ~35us first-collective ncfw cost so the real
  AllReduce on the critical tail runs at the ~10us floor.
"""
import sys

sys.path.insert(0, "/opt/trn_rl_repo")
import numpy as np

B, T, H, N2 = 4, 64, 1280, 32
DIN = 224 * 224 * 3  # 150528
R_HID, NCLS = 64, 60
NCORES = 8
KS = DIN // NCORES   # 18816
KT = KS // 128       # 147
MT = H // 128        # 10
TOK = B * T          # 256
GO = H // NCORES     # 160 GLU output channels per core
HGO = GO // 2        # 80
CH = 3               # k-tiles per DMA chunk
NCHUNK = KT // CH    # 49

_compiled = None


def _build():
    import concourse.bacc as bacc
    import concourse.mybir as mybir
    import concourse.tile as tile
    from concourse.tile import add_dep_helper

    f32 = mybir.dt.float32
    f32r = mybir.dt.float32r
    AF = mybir.ActivationFunctionType
    OP = mybir.AluOpType
    RG = [list(range(NCORES))]

    nc = bacc.Bacc("TRN2", target_bir_lowering=False, debug=False,
                   num_devices=NCORES)

    d_xT = nc.dram_tensor("xT", [128, KT * TOK], f32r, kind="ExternalInput").ap()
    d_wT = nc.dram_tensor("wT", [128, KT * H], f32r, kind="ExternalInput").ap()
    d_bb = nc.dram_tensor("bb", [H, 1], f32, kind="ExternalInput").ap()
    d_logdt = nc.dram_tensor("logdt", [H, 1], f32, kind="ExternalInput").ap()
    d_logA = nc.dram_tensor("logA", [H, N2], f32, kind="ExternalInput").ap()
    d_C = nc.dram_tensor("Cmat", [H, N2], f32, kind="ExternalInput").ap()
    d_D = nc.dram_tensor("Dvec", [H, 1], f32, kind="ExternalInput").ap()
    d_rev = nc.dram_tensor("rev", [128, T * N2], f32, kind="ExternalInput").ap()
    d_wcT = nc.dram_tensor("wcT", [H, 4 * HGO], f32r, kind="ExternalInput").ap()
    d_bc = nc.dram_tensor("bc", [4 * HGO, 1], f32, kind="ExternalInput").ap()
    d_w1T = nc.dram_tensor("w1T", [H, R_HID], f32r, kind="ExternalInput").ap()
    d_b1 = nc.dram_tensor("b1", [R_HID, 1], f32, kind="ExternalInput").ap()
    d_w2T = nc.dram_tensor("w2T", [R_HID, NCLS], f32r, kind="ExternalInput").ap()
    d_b2 = nc.dram_tensor("b2", [NCLS, 1], f32, kind="ExternalInput").ap()
    d_out = nc.dram_tensor("out", [NCLS, B], f32, kind="ExternalOutput").ap()

    with tile.TileContext(nc) as tc:
        with tc.tile_pool(name="cpool", bufs=1) as cpool, \
             tc.tile_pool(name="dram", bufs=1, space="DRAM") as dp:
            py_in = dp.tile([H, B], f32, tag="py_in")
            py_out = dp.tile([H, B], f32, tag="py_out", addr_space="Shared")
            glu_in = dp.tile([GO, B], f32r, tag="glu_in")
            glu_out = dp.tile([H, B], f32r, tag="glu_out", addr_space="Shared")

            # ---- collective warmup: tiny AllReduce absorbs ncfw first-call
            warm_in = dp.tile([NCORES, B], f32, tag="warm_in")
            warm_out = dp.tile([NCORES, B], f32, tag="warm_out",
                               addr_space="Shared")
            wz = cpool.tile([NCORES, B], f32, tag="wz")
            nc.vector.memset(wz[:], 0.0)
            nc.scalar.dma_start(warm_in[:, :], wz[:])
            nc.gpsimd.collective_compute(
                "AllReduce", OP.add, replica_groups=RG,
                ins=[warm_in.opt()], outs=[warm_out.opt()])

            # ---- Phase B: build reversed S4D kernels k_rev (per 128-chan tile)
            # k_rev[h, t] = 2 * sum_n C[h,n]*(exp(dtA[h,n])-1)/A[h,n]
            #                        * exp(dtA[h,n]*(63-t))
            rev_t = cpool.tile([128, T * N2], f32, tag="rev")
            nc.scalar.dma_start(rev_t[:], d_rev)
            krev, bbs, Ds = [], [], []
            with tc.tile_pool(name="kb", bufs=1) as kb:
                for m in range(MT):
                    sl = slice(m * 128, (m + 1) * 128)
                    t_logdt = kb.tile([128, 1], f32, tag="logdt")
                    t_logA = kb.tile([128, N2], f32, tag="logA")
                    t_C = kb.tile([128, N2], f32, tag="C")
                    nc.scalar.dma_start(t_logdt[:], d_logdt[sl, :])
                    nc.scalar.dma_start(t_logA[:], d_logA[sl, :])
                    nc.scalar.dma_start(t_C[:], d_C[sl, :])
                    t_bb = cpool.tile([128, 1], f32, tag=f"bb{m}")
                    t_D = cpool.tile([128, 1], f32, tag=f"D{m}")
                    nc.scalar.dma_start(t_bb[:], d_bb[sl, :])
                    nc.scalar.dma_start(t_D[:], d_D[sl, :])
                    bbs.append(t_bb)
                    Ds.append(t_D)
                    # bb/8: bias is added once globally via the AllReduce sum
                    nc.vector.tensor_scalar_mul(t_bb[:], t_bb[:], 1.0 / NCORES)

                    t_dt = kb.tile([128, 1], f32, tag="dt")
                    nc.scalar.activation(t_dt[:], t_logdt[:], AF.Exp)
                    negA = kb.tile([128, N2], f32, tag="negA")
                    nc.scalar.activation(negA[:], t_logA[:], AF.Exp)
                    dtA = kb.tile([128, N2], f32, tag="dtA")
                    # dtA = A*dt = -(negA*dt)
                    nc.vector.tensor_scalar(dtA[:], negA[:], t_dt[:], -1.0,
                                            OP.mult, OP.mult)
                    expdtA = kb.tile([128, N2], f32, tag="expdtA")
                    nc.scalar.activation(expdtA[:], dtA[:], AF.Exp)
                    recipA = kb.tile([128, N2], f32, tag="recipA")
                    nc.vector.reciprocal(recipA[:], negA[:])
                    # cb2 = 2*C*(exp(dtA)-1)/A = [ (expdtA-1)*(-2) ] * C * (1/negA)
                    cb2 = kb.tile([128, N2], f32, tag="cb2")
                    nc.vector.tensor_scalar(cb2[:], expdtA[:], 1.0, -2.0,
                                            OP.subtract, OP.mult)
                    nc.vector.tensor_mul(cb2[:], cb2[:], t_C[:])
                    nc.vector.tensor_mul(cb2[:], cb2[:], recipA[:])
                    # G[p, t, n] = rev[t] * dtA[p, n]
                    G = kb.tile([128, T * N2], f32, tag="G")
                    G3 = G[:].rearrange("p (t n) -> p t n", t=T)
                    nc.vector.tensor_tensor(
                        G3, rev_t[:].rearrange("p (t n) -> p t n", t=T),
                        dtA[:].unsqueeze(1).broadcast_to((128, T, N2)),
                        op=OP.mult)
                    expG = kb.tile([128, T * N2], f32, tag="expG")
                    nc.scalar.activation(expG[:], G[:], AF.Exp)
                    nc.vector.tensor_tensor(
                        expG[:].rearrange("p (t n) -> p t n", t=T),
                        expG[:].rearrange("p (t n) -> p t n", t=T),
                        cb2[:].unsqueeze(1).broadcast_to((128, T, N2)),
                        op=OP.mult)
                    kr = cpool.tile([128, T], f32, tag=f"krev{m}")
                    nc.vector.reduce_sum(
                        kr[:], expG[:].rearrange("p (t n) -> p t n", t=T),
                        axis=mybir.AxisListType.X)
                    krev.append(kr)

            # ---- preload epilogue weights (overlaps the big matmul stream)
            wcs, w1s = [], []
            for k in range(MT):
                t_wc = cpool.tile([128, 4 * HGO], f32r, tag=f"wc{k}",
                                  name=f"wc{k}")
                nc.sync.dma_start(t_wc[:], d_wcT[k * 128:(k + 1) * 128, :])
                wcs.append(t_wc)
                t_w1 = cpool.tile([128, R_HID], f32r, tag=f"w1_{k}",
                                  name=f"w1_{k}")
                nc.sync.dma_start(t_w1[:], d_w1T[k * 128:(k + 1) * 128, :])
                w1s.append(t_w1)
            bcg = []
            for j in range(4):
                t_bc = cpool.tile([HGO, 1], f32, tag=f"bc{j}", name=f"bc{j}")
                nc.scalar.dma_start(t_bc[:], d_bc[j * HGO:(j + 1) * HGO, :])
                bcg.append(t_bc)
            w2 = cpool.tile([R_HID, NCLS], f32r, tag="w2")
            nc.scalar.dma_start(w2[:], d_w2T)
            t_b1 = cpool.tile([R_HID, 1], f32, tag="b1")
            nc.scalar.dma_start(t_b1[:], d_b1)
            t_b2 = cpool.tile([NCLS, 1], f32, tag="b2")
            nc.scalar.dma_start(t_b2[:], d_b2)

            # ---- Phase A: big matmul  u^T(partial) = wT_slice.T @ xT_slice
            with tc.tile_pool(name="wp", bufs=4) as wp, \
                 tc.tile_pool(name="xp", bufs=4) as xp, \
                 tc.tile_pool(name="psA", bufs=1, space="PSUM") as pA, \
                 tc.tile_pool(name="ev", bufs=2) as ev:
                psu = [pA.tile([128, 512], f32, tag=f"u{j}", name=f"u{j}")
                       for j in range(5)]

                # PE warmup: ~6us of zero matmuls into a scratch bank lifts
                # the HAM clock gate while the first weight chunk streams in.
                warm_ps = pA.tile([128, 512], f32, tag="warmps")
                warm_z = cpool.tile([128, 512], f32, tag="warmz")
                warm_w = cpool.tile([128, 128], f32r, tag="warmw")
                warm_x = cpool.tile([128, 512], f32r, tag="warmx")
                nc.vector.memset(warm_z[:], 0.0)
                nc.vector.tensor_copy(warm_w[:], warm_z[:, 0:128])
                nc.vector.tensor_copy(warm_x[:], warm_z[:])
                for _ in range(16):
                    nc.tensor.matmul(warm_ps[:], warm_w[:], warm_x[:],
                                     start=True, stop=True)

                mm_first, mm_last = {}, {}
                for kc in range(NCHUNK):
                    wt = wp.tile([128, CH * H], f32r, tag="wt")
                    xt = xp.tile([128, CH * TOK], f32r, tag="xt")
                    wq = nc.sync if kc % 2 == 0 else nc.scalar
                    xq = nc.scalar if kc % 2 == 0 else nc.sync
                    wq.dma_start(
                        wt[:], d_wT[:, kc * CH * H:(kc + 1) * CH * H])
                    xq.dma_start(
                        xt[:], d_xT[:, kc * CH * TOK:(kc + 1) * CH * TOK])
                    for j_in in range(CH):
                        k = kc * CH + j_in
                        for m in range(MT):
                            j, half = divmod(m, 2)
                            # two 256-wide accumulation groups share each 2KB
                            # PSUM bank: only the even half emits start
                            # (zeroing the whole bank region), only the odd
                            # half emits stop.
                            inst = nc.tensor.matmul(
                                psu[j][:, half * 256:(half + 1) * 256],
                                wt[:, j_in * H + m * 128:
                                   j_in * H + (m + 1) * 128],
                                xt[:, j_in * TOK:(j_in + 1) * TOK],
                                start=(k == 0 and half == 0),
                                stop=(k == KT - 1 and half == 1))
                            if k == 0:
                                mm_first[m] = inst
                            if k == KT - 1:
                                mm_last[m] = inst
                for j in range(5):
                    add_dep_helper(mm_first[2 * j + 1].ins, mm_first[2 * j].ins,
                                   reason="psum zero-region start order")
                    add_dep_helper(mm_last[2 * j + 1].ins, mm_last[2 * j].ins,
                                   reason="psum zero-region stop order")

                # ---- Phase C: u + bb/8, conv with k_rev, D-skip, partial y
                y_all = ev.tile([128, MT * B], f32, tag="yall")
                for m in range(MT):
                    j, half = divmod(m, 2)
                    u_sb = ev.tile([128, TOK], f32, tag="usb")
                    nc.scalar.activation(u_sb[:],
                                         psu[j][:, half * 256:(half + 1) * 256],
                                         AF.Identity, bias=bbs[m][:])
                    u3 = u_sb[:].rearrange("p (b t) -> p b t", b=B)
                    pr = ev.tile([128, TOK], f32, tag="pr")
                    nc.vector.tensor_tensor(
                        pr[:].rearrange("p (b t) -> p b t", b=B), u3,
                        krev[m][:].unsqueeze(1).broadcast_to((128, B, T)),
                        op=OP.mult)
                    y_m = y_all[:, m * B:(m + 1) * B]
                    nc.vector.reduce_sum(
                        y_m, pr[:].rearrange("p (b t) -> p b t", b=B),
                        axis=mybir.AxisListType.X)
                    dsk = ev.tile([128, B], f32, tag="dsk")
                    nc.vector.tensor_scalar(dsk[:], u3[:, :, T - 1], Ds[m][:],
                                            None, OP.mult)
                    nc.vector.tensor_add(y_m, y_m, dsk[:])
                # single DMA scatters y_all[p, m, b] -> py_in[m*128+p, b]
                nc.scalar.dma_start(
                    py_in.rearrange("(m p) b -> p m b", p=128),
                    y_all[:].rearrange("p (m b) -> p m b", m=MT))

                nc.gpsimd.collective_compute(
                    "AllReduce", OP.add, replica_groups=RG,
                    ins=[py_in.opt()], outs=[py_out.opt()])

            # ---- Phase D: GELU + GLU (sharded: 160 channels per core)
            with tc.tile_pool(name="de", bufs=1) as de, \
                 tc.tile_pool(name="psB", bufs=1, space="PSUM") as pB:
                # one DMA gathers all of y_last: yg_all[p, m, b] = py_out[m*128+p, b]
                yg_raw = de.tile([128, MT * B], f32, tag="ygraw")
                src_y = py_out.rearrange("(m p) b -> p m b", p=128)
                nc.scalar.dma_start(
                    yg_raw[:].rearrange("p (m b) -> p m b", m=MT), src_y)
                yg_all = de.tile([128, MT * B], f32r, tag="ygall")
                nc.scalar.activation(yg_all[:], yg_raw[:], AF.Gelu)
                yg = [yg_all[:, m * B:(m + 1) * B] for m in range(MT)]
                z = [pB.tile([HGO, B], f32, tag=f"z{j}", name=f"z{j}")
                     for j in range(4)]
                for k in range(MT):
                    for j in range(4):
                        nc.tensor.matmul(z[j][:],
                                         wcs[k][:, j * HGO:(j + 1) * HGO],
                                         yg[k], start=(k == 0),
                                         stop=(k == MT - 1))
                a0 = de.tile([HGO, B], f32, tag="a0")
                a1 = de.tile([HGO, B], f32, tag="a1")
                s0 = de.tile([HGO, B], f32, tag="s0")
                s1 = de.tile([HGO, B], f32, tag="s1")
                nc.scalar.activation(a0[:], z[0][:], AF.Identity, bias=bcg[0][:])
                nc.scalar.activation(a1[:], z[1][:], AF.Identity, bias=bcg[1][:])
                nc.scalar.activation(s0[:], z[2][:], AF.Sigmoid, bias=bcg[2][:])
                nc.scalar.activation(s1[:], z[3][:], AF.Sigmoid, bias=bcg[3][:])
                g0 = de.tile([HGO, B], f32r, tag="g0")
                g1 = de.tile([HGO, B], f32r, tag="g1")
                nc.vector.tensor_mul(g0[:], a0[:], s0[:])
                nc.vector.tensor_mul(g1[:], a1[:], s1[:])
                nc.scalar.dma_start(glu_in[0:HGO, :], g0[:])
                nc.scalar.dma_start(glu_in[HGO:GO, :], g1[:])

                nc.gpsimd.collective_compute(
                    "AllGather", OP.bypass, replica_groups=RG,
                    ins=[glu_in.opt()], outs=[glu_out.opt()])

                # ---- Phase E: readout MLP
                ps_h = pB.tile([R_HID, B], f32, tag="ph")
                gf_all = de.tile([128, MT * B], f32r, tag="gfall")
                src_g = glu_out.rearrange("(m p) b -> p m b", p=128)
                nc.scalar.dma_start(
                    gf_all[:].rearrange("p (m b) -> p m b", m=MT), src_g)
                for k in range(MT):
                    nc.tensor.matmul(ps_h[:], w1s[k][:],
                                     gf_all[:, k * B:(k + 1) * B],
                                     start=(k == 0), stop=(k == MT - 1))
                h1 = de.tile([R_HID, B], f32r, tag="h1")
                nc.scalar.activation(h1[:], ps_h[:], AF.Relu, bias=t_b1[:])
                ps_o = pB.tile([NCLS, B], f32, tag="po")
                nc.tensor.matmul(ps_o[:], w2[:], h1[:], start=True, stop=True)
                o_sb = de.tile([NCLS, B], f32, tag="osb")
                nc.scalar.activation(o_sb[:], ps_o[:], AF.Identity,
                                     bias=t_b2[:])
                nc.scalar.dma_start(d_out, o_sb[:])

    nc.compile()
    return nc


def _prep_inputs(inputs):
    x = np.asarray(inputs["x"], dtype=np.float32)
    Wb = np.asarray(inputs["Wb"], dtype=np.float32)
    bb = np.asarray(inputs["bb"], dtype=np.float32)
    log_dt = np.asarray(inputs["log_dt"], dtype=np.float32)
    C = np.asarray(inputs["C"], dtype=np.float32)
    logA = np.asarray(inputs["log_A_real"], dtype=np.float32)
    D = np.asarray(inputs["D"], dtype=np.float32)
    Wc = np.asarray(inputs["Wc"], dtype=np.float32)
    bc = np.asarray(inputs["bc"], dtype=np.float32)
    W1 = np.asarray(inputs["W1"], dtype=np.float32)
    b1 = np.asarray(inputs["b1"], dtype=np.float32)
    W2 = np.asarray(inputs["W2"], dtype=np.float32)
    b2 = np.asarray(inputs["b2"], dtype=np.float32)

    xT = np.ascontiguousarray(x.reshape(TOK, DIN).T)     # (DIN, 256)
    wT = np.ascontiguousarray(Wb.T)                      # (DIN, 1280)
    WcT = np.ascontiguousarray(Wc.T)                     # (1280, 2560)
    W1T = np.ascontiguousarray(W1.T)                     # (1280, 64)
    W2T = np.ascontiguousarray(W2.T)                     # (64, 60)
    rev = np.arange(T - 1, -1, -1, dtype=np.float32)     # 63 - t
    rev_full = np.ascontiguousarray(
        np.broadcast_to(np.repeat(rev, N2), (128, T * N2)))

    shared = {
        "bb": bb.reshape(H, 1), "logdt": log_dt.reshape(H, 1),
        "logA": logA, "Cmat": C, "Dvec": D.reshape(H, 1), "rev": rev_full,
        "w1T": W1T, "b1": b1.reshape(R_HID, 1),
        "w2T": W2T, "b2": b2.reshape(NCLS, 1),
    }
    in_maps = []
    for i in range(NCORES):
        klo = i * KS
        go = i * GO
        # partition-major repack: arr_p[p, k, :] = arr[k*128+p, :] so each
        # DMA chunk reads one large contiguous block per partition
        wTp = np.ascontiguousarray(
            wT[klo:klo + KS].reshape(KT, 128, H).transpose(1, 0, 2)
        ).reshape(128, KT * H)
        xTp = np.ascontiguousarray(
            xT[klo:klo + KS].reshape(KT, 128, TOK).transpose(1, 0, 2)
        ).reshape(128, KT * TOK)
        wcT_sl = np.ascontiguousarray(np.concatenate(
            [WcT[:, go:go + HGO], WcT[:, go + HGO:go + GO],
             WcT[:, H + go:H + go + HGO], WcT[:, H + go + HGO:H + go + GO]],
            axis=1))
        bc_sl = np.ascontiguousarray(np.concatenate(
            [bc[go:go + HGO], bc[go + HGO:go + GO],
             bc[H + go:H + go + HGO],
             bc[H + go + HGO:H + go + GO]]).reshape(4 * HGO, 1))
        in_maps.append({
            "xT": xTp, "wT": wTp,
            "wcT": wcT_sl, "bc": bc_sl, **shared,
        })
    return in_maps


def kernel(**inputs):
    global _compiled
    if _compiled is None:
        _compiled = _build()
    nc = _compiled
    in_maps = _prep_inputs(inputs)
    from concourse import bass_utils
    res = bass_utils.run_bass_kernel_spmd(nc, in_maps,
                                          core_ids=list(range(NCORES)))
    out = res.results[0]["out"]  # (NCLS, B)
    return np.ascontiguousarray(out.T).astype(np.float32)


# revision 17
# speedup vs baseline: 1.2179x; 1.2179x over previous
"""Trainium2 Bass kernel for nn_BottleneckS4D (8-core SPMD).

Strategy (self-contained, hardcoded):
  The reference is  u = x_flat @ Wb.T + bb  (256 x 150528 @ 150528 x 1280,
  770MB weight) followed by an S4D block whose output is only consumed at
  the LAST timestep (readout takes y[:, -1, :]), so the FFT convolution
  collapses to a per-channel dot product over time with the reversed S4D
  kernel, and everything downstream is tiny.

  Sharding: split the CONTRACTION dim D_IN=150528 across the 8 cores
  (18816 each). Each core streams its 96MB weight slice + 19MB x slice
  once (total HBM traffic = one pass over the data, the minimum), and
  computes a partial u^T (1280, 256) in PSUM with fp32r matmuls (full
  bf16-rate, ~1e-4 relative error). The S4D conv is linear in u, so each
  core reduces its partial u to a partial y_last (1280, 4); a tiny
  ReduceScatter sums it (each core gets its 160 channels), GELU runs on
  the shard, an AllGather rebuilds the full y_gelu, the GLU 1x1 conv is
  computed for 160 channels/core and AllGathered, and the readout MLP
  runs on every core; core 0's output is returned.

  Perf details: weights/x are host-repacked to partition-major layout
  (wTp[p, k, :] = wT[k*128+p, :]) so each DMA chunk moves CH k-tiles with
  one large contiguous descriptor per partition; wt/xt chunks alternate
  between the sync and scalar HWDGE queues; all small tensors arrive in
  3 packed partition-major buffers (3 big DMAs instead of ~80 tiny ones,
  which otherwise jam the DMA rings for ~75us at kernel start); chunk 0
  and a PE warmup burst are emitted first so the PE's HAM clock gate
  lifts while the first chunks stream; PSUM accumulation restarts at
  k=KSPLIT so the conv of the first half overlaps the remaining stream;
  a dummy AllReduce early in the kernel absorbs the ncfw
  first-collective cost.
"""
import sys

sys.path.insert(0, "/opt/trn_rl_repo")
import numpy as np

B, T, H, N2 = 4, 64, 1280, 32
DIN = 224 * 224 * 3  # 150528
R_HID, NCLS = 64, 60
NCORES = 8
KS = DIN // NCORES   # 18816
KT = KS // 128       # 147
MT = H // 128        # 10
TOK = B * T          # 256
GO = H // NCORES     # 160 GLU output channels per core
HGO = GO // 2        # 80
CH = 3               # k-tiles per DMA chunk
NCHUNK = KT // CH    # 49
KSPLIT = 99          # conv of k<KSPLIT overlaps the remaining stream
PA_COLS = T * N2 + 3 * MT + 2 * MT * N2    # 2718
PB_COLS = MT * 4 * HGO + MT * R_HID + NCLS  # 3900

_compiled = None


def _build():
    import concourse.bacc as bacc
    import concourse.mybir as mybir
    import concourse.tile as tile
    from concourse.tile import add_dep_helper

    f32 = mybir.dt.float32
    f32r = mybir.dt.float32r
    AF = mybir.ActivationFunctionType
    OP = mybir.AluOpType
    RG = [list(range(NCORES))]

    nc = bacc.Bacc("TRN2", target_bir_lowering=False, debug=False,
                   num_devices=NCORES)

    d_xT = nc.dram_tensor("xT", [128, KT * TOK], f32r, kind="ExternalInput").ap()
    d_wT = nc.dram_tensor("wT", [128, KT * H], f32r, kind="ExternalInput").ap()
    # packedA cols: rev(2048) | logdt(10) | bb(10) | D(10) | logA(320) | C(320)
    d_pa = nc.dram_tensor("packedA", [128, PA_COLS], f32,
                          kind="ExternalInput").ap()
    # packedB cols: wcT(10*320) | w1T(10*64) | w2T(60, rows padded to 128)
    d_pb = nc.dram_tensor("packedB", [128, PB_COLS], f32r,
                          kind="ExternalInput").ap()
    # packedS cols: b1 | b2 | bc(4 cols, 80 rows)
    d_ps = nc.dram_tensor("packedS", [128, 6], f32, kind="ExternalInput").ap()
    d_out = nc.dram_tensor("out", [NCLS, B], f32, kind="ExternalOutput").ap()

    with tile.TileContext(nc) as tc:
        with tc.tile_pool(name="cpool", bufs=1) as cpool, \
             tc.tile_pool(name="dram", bufs=1, space="DRAM") as dp, \
             tc.tile_pool(name="wp", bufs=4) as wp, \
             tc.tile_pool(name="xp", bufs=4) as xp, \
             tc.tile_pool(name="ev", bufs=2) as ev:
            py_in = dp.tile([H, B], f32, tag="py_in")
            py_rs = dp.tile([GO, B], f32, tag="py_rs")
            yg_in = dp.tile([GO, B], f32r, tag="yg_in")
            yg_full = dp.tile([H, B], f32r, tag="yg_full", addr_space="Shared")
            glu_in = dp.tile([GO, B], f32r, tag="glu_in")
            glu_out = dp.tile([H, B], f32r, tag="glu_out", addr_space="Shared")

            # ---- collective warmup: tiny AllReduce absorbs the ncfw
            # first-collective cost concurrently with the matmul stream
            warm_in = dp.tile([NCORES, B], f32, tag="warm_in")
            warm_out = dp.tile([NCORES, B], f32, tag="warm_out",
                               addr_space="Shared")
            wz = cpool.tile([NCORES, B], f32, tag="wz")
            nc.vector.memset(wz[:], 0.0)
            nc.scalar.dma_start(warm_in[:, :], wz[:])
            nc.gpsimd.collective_compute(
                "AllReduce", OP.add, replica_groups=RG,
                ins=[warm_in.opt()], outs=[warm_out.opt()])

            with tc.tile_pool(name="psA", bufs=1, space="PSUM") as pA:
                # ---- PE warmup burst + first chunks, emitted ahead of rest
                psu = [pA.tile([128, 512], f32, tag=f"u{j}", name=f"u{j}")
                       for j in range(5)]
                warm_ps = pA.tile([128, 512], f32, tag="warmps")
                warm_z = cpool.tile([128, 512], f32, tag="warmz")
                warm_w = cpool.tile([128, 128], f32r, tag="warmw")
                warm_x = cpool.tile([128, 512], f32r, tag="warmx")
                nc.vector.memset(warm_z[:], 0.0)
                nc.vector.tensor_copy(warm_w[:], warm_z[:, 0:128])
                nc.vector.tensor_copy(warm_x[:], warm_z[:])
                for _ in range(16):
                    nc.tensor.matmul(warm_ps[:], warm_w[:], warm_x[:],
                                     start=True, stop=True)

                mm_marks = {}

                def do_chunk(kc):
                    wt = wp.tile([128, CH * H], f32r, tag="wt", name="wt")
                    xt = xp.tile([128, CH * TOK], f32r, tag="xt", name="xt")
                    wq = nc.sync if kc % 2 == 0 else nc.scalar
                    xq = nc.scalar if kc % 2 == 0 else nc.sync
                    wq.dma_start(wt[:], d_wT[:, kc * CH * H:(kc + 1) * CH * H])
                    xq.dma_start(xt[:],
                                 d_xT[:, kc * CH * TOK:(kc + 1) * CH * TOK])
                    for j_in in range(CH):
                        k = kc * CH + j_in
                        for m in range(MT):
                            j, half = divmod(m, 2)
                            # two 256-wide accumulation groups share each 2KB
                            # PSUM bank: only the even half emits start
                            # (zeroing the whole bank region), only the odd
                            # half emits stop. Accumulation restarts at
                            # k=KSPLIT for the split conv.
                            inst = nc.tensor.matmul(
                                psu[j][:, half * 256:(half + 1) * 256],
                                wt[:, j_in * H + m * 128:
                                   j_in * H + (m + 1) * 128],
                                xt[:, j_in * TOK:(j_in + 1) * TOK],
                                start=(k in (0, KSPLIT) and half == 0),
                                stop=(k in (KSPLIT - 1, KT - 1) and half == 1))
                            if k in (0, KSPLIT - 1, KSPLIT, KT - 1):
                                mm_marks[(k, m)] = inst

                do_chunk(0)
                do_chunk(1)

                # ---- Phase B: packed smalls + reversed S4D kernels k_rev
                # k_rev[h, t] = 2 * sum_n C[h,n]*(exp(dtA[h,n])-1)/A[h,n]
                #                        * exp(dtA[h,n]*(63-t))
                pa = cpool.tile([128, PA_COLS], f32, tag="pa")
                nc.scalar.dma_start(pa[:], d_pa)
                psmall = cpool.tile([128, 6], f32, tag="psmall")
                nc.scalar.dma_start(psmall[:], d_ps)
                O_LD, O_BB, O_D = T * N2, T * N2 + MT, T * N2 + 2 * MT
                O_LA, O_C = T * N2 + 3 * MT, T * N2 + 3 * MT + MT * N2
                rev3 = pa[:, 0:T * N2].rearrange("p (t n) -> p t n", t=T)
                bb8 = cpool.tile([128, MT], f32, tag="bb8")
                krev, bbs, Ds = [], [], []
                with tc.tile_pool(name="kb", bufs=1) as kb:
                    for m in range(MT):
                        t_logdt = pa[:, O_LD + m:O_LD + m + 1]
                        t_logA = pa[:, O_LA + m * N2:O_LA + (m + 1) * N2]
                        t_C = pa[:, O_C + m * N2:O_C + (m + 1) * N2]
                        # bb/8: bias summed once via the cross-core reduce
                        nc.vector.tensor_scalar_mul(
                            bb8[:, m:m + 1], pa[:, O_BB + m:O_BB + m + 1],
                            1.0 / NCORES)
                        bbs.append(bb8[:, m:m + 1])
                        Ds.append(pa[:, O_D + m:O_D + m + 1])

                        t_dt = kb.tile([128, 1], f32, tag="dt")
                        nc.scalar.activation(t_dt[:], t_logdt, AF.Exp)
                        negA = kb.tile([128, N2], f32, tag="negA")
                        nc.scalar.activation(negA[:], t_logA, AF.Exp)
                        dtA = kb.tile([128, N2], f32, tag="dtA")
                        # dtA = A*dt = -(negA*dt)
                        nc.vector.tensor_scalar(dtA[:], negA[:], t_dt[:], -1.0,
                                                OP.mult, OP.mult)
                        expdtA = kb.tile([128, N2], f32, tag="expdtA")
                        nc.scalar.activation(expdtA[:], dtA[:], AF.Exp)
                        recipA = kb.tile([128, N2], f32, tag="recipA")
                        nc.vector.reciprocal(recipA[:], negA[:])
                        # cb2 = 2*C*(exp(dtA)-1)/A = [(expdtA-1)*(-2)]*C/negA
                        cb2 = kb.tile([128, N2], f32, tag="cb2")
                        nc.vector.tensor_scalar(cb2[:], expdtA[:], 1.0, -2.0,
                                                OP.subtract, OP.mult)
                        nc.vector.tensor_mul(cb2[:], cb2[:], t_C)
                        nc.vector.tensor_mul(cb2[:], cb2[:], recipA[:])
                        # G[p, t, n] = rev[t] * dtA[p, n]
                        G = kb.tile([128, T * N2], f32, tag="G")
                        G3 = G[:].rearrange("p (t n) -> p t n", t=T)
                        nc.vector.tensor_tensor(
                            G3, rev3,
                            dtA[:].unsqueeze(1).broadcast_to((128, T, N2)),
                            op=OP.mult)
                        expG = kb.tile([128, T * N2], f32, tag="expG")
                        nc.scalar.activation(expG[:], G[:], AF.Exp)
                        nc.vector.tensor_tensor(
                            expG[:].rearrange("p (t n) -> p t n", t=T),
                            expG[:].rearrange("p (t n) -> p t n", t=T),
                            cb2[:].unsqueeze(1).broadcast_to((128, T, N2)),
                            op=OP.mult)
                        kr = cpool.tile([128, T], f32, tag=f"krev{m}")
                        nc.vector.reduce_sum(
                            kr[:], expG[:].rearrange("p (t n) -> p t n", t=T),
                            axis=mybir.AxisListType.X)
                        krev.append(kr)

                y1 = ev.tile([128, MT * B], f32, tag="y1")
                y2 = ev.tile([128, MT * B], f32, tag="y2")

                def conv_pass(y_dst, first):
                    # y_dst[:, m*B:] = conv(u_part) + D-skip (+bias if first)
                    for m in range(MT):
                        j, half = divmod(m, 2)
                        u_sb = ev.tile([128, TOK], f32, tag="usb", name="usb")
                        nc.scalar.activation(
                            u_sb[:], psu[j][:, half * 256:(half + 1) * 256],
                            AF.Identity, bias=(bbs[m] if first else 0.0))
                        u3 = u_sb[:].rearrange("p (b t) -> p b t", b=B)
                        pr = ev.tile([128, TOK], f32, tag="pr", name="pr")
                        nc.vector.tensor_tensor(
                            pr[:].rearrange("p (b t) -> p b t", b=B), u3,
                            krev[m][:].unsqueeze(1).broadcast_to((128, B, T)),
                            op=OP.mult)
                        y_m = y_dst[:, m * B:(m + 1) * B]
                        nc.vector.reduce_sum(
                            y_m, pr[:].rearrange("p (b t) -> p b t", b=B),
                            axis=mybir.AxisListType.X)
                        dsk = ev.tile([128, B], f32, tag="dsk", name="dsk")
                        nc.vector.tensor_scalar(dsk[:], u3[:, :, T - 1],
                                                Ds[m], None, OP.mult)
                        nc.vector.tensor_add(y_m, y_m, dsk[:])

                # ---- Phase A: remaining chunks; first conv overlaps stream
                for kc in range(2, NCHUNK):
                    do_chunk(kc)
                    if kc == KSPLIT // CH - 1:  # chunk 32 completes k=0..98
                        conv_pass(y1, True)
                for j in range(5):
                    for kk in (0, KSPLIT - 1, KSPLIT, KT - 1):
                        add_dep_helper(mm_marks[(kk, 2 * j + 1)].ins,
                                       mm_marks[(kk, 2 * j)].ins,
                                       reason="psum zero-region order")

                # epilogue weights: one packed DMA, needed only in phase D
                pb_t = cpool.tile([128, PB_COLS], f32r, tag="pbt")
                nc.sync.dma_start(pb_t[:], d_pb)
                wcs = [pb_t[:, k * 4 * HGO:(k + 1) * 4 * HGO]
                       for k in range(MT)]
                O_W1 = MT * 4 * HGO
                w1s = [pb_t[:, O_W1 + k * R_HID:O_W1 + (k + 1) * R_HID]
                       for k in range(MT)]
                w2 = pb_t[0:R_HID, O_W1 + MT * R_HID:O_W1 + MT * R_HID + NCLS]
                t_b1 = psmall[0:R_HID, 0:1]
                t_b2 = psmall[0:NCLS, 1:2]
                bcg = [psmall[0:HGO, 2 + j:3 + j] for j in range(4)]

                # ---- Phase C tail: second conv pass, combine, scatter out
                conv_pass(y2, False)
                yf = ev.tile([128, MT * B], f32, tag="yf")
                nc.vector.tensor_add(yf[:], y1[:], y2[:])
                # single DMA scatters yf[p, m, b] -> py_in[m*128+p, b]
                nc.scalar.dma_start(
                    py_in.rearrange("(m p) b -> p m b", p=128),
                    yf[:].rearrange("p (m b) -> p m b", m=MT))

            # ReduceScatter: rank i receives summed rows [i*160, (i+1)*160)
            nc.gpsimd.collective_compute(
                "ReduceScatter", OP.add, replica_groups=RG,
                ins=[py_in.opt()], outs=[py_rs.opt()])

            # ---- Phase D: GELU on own shard + AllGather + GLU
            with tc.tile_pool(name="de", bufs=1) as de, \
                 tc.tile_pool(name="psB", bufs=1, space="PSUM") as pB:
                yra = de.tile([128, B], f32, tag="yra")
                yrb = de.tile([GO - 128, B], f32, tag="yrb")
                nc.scalar.dma_start(yra[:], py_rs[0:128, :])
                nc.scalar.dma_start(yrb[:], py_rs[128:GO, :])
                gya = de.tile([128, B], f32r, tag="gya")
                gyb = de.tile([GO - 128, B], f32r, tag="gyb")
                nc.scalar.activation(gya[:], yra[:], AF.Gelu)
                nc.scalar.activation(gyb[:], yrb[:], AF.Gelu)
                nc.scalar.dma_start(yg_in[0:128, :], gya[:])
                nc.scalar.dma_start(yg_in[128:GO, :], gyb[:])
                nc.gpsimd.collective_compute(
                    "AllGather", OP.bypass, replica_groups=RG,
                    ins=[yg_in.opt()], outs=[yg_full.opt()])
                # one DMA gathers y_gelu: yg_all[p, m, b] = yg_full[m*128+p, b]
                yg_all = de.tile([128, MT * B], f32r, tag="ygall")
                src_y = yg_full.rearrange("(m p) b -> p m b", p=128)
                nc.scalar.dma_start(
                    yg_all[:].rearrange("p (m b) -> p m b", m=MT), src_y)
                yg = [yg_all[:, m * B:(m + 1) * B] for m in range(MT)]
                z = [pB.tile([HGO, B], f32, tag=f"z{j}", name=f"z{j}")
                     for j in range(4)]
                for k in range(MT):
                    for j in range(4):
                        nc.tensor.matmul(z[j][:],
                                         wcs[k][:, j * HGO:(j + 1) * HGO],
                                         yg[k], start=(k == 0),
                                         stop=(k == MT - 1))
                a0 = de.tile([HGO, B], f32, tag="a0")
                a1 = de.tile([HGO, B], f32, tag="a1")
                s0 = de.tile([HGO, B], f32, tag="s0")
                s1 = de.tile([HGO, B], f32, tag="s1")
                nc.scalar.activation(a0[:], z[0][:], AF.Identity, bias=bcg[0])
                nc.scalar.activation(a1[:], z[1][:], AF.Identity, bias=bcg[1])
                nc.scalar.activation(s0[:], z[2][:], AF.Sigmoid, bias=bcg[2])
                nc.scalar.activation(s1[:], z[3][:], AF.Sigmoid, bias=bcg[3])
                g0 = de.tile([HGO, B], f32r, tag="g0")
                g1 = de.tile([HGO, B], f32r, tag="g1")
                nc.vector.tensor_mul(g0[:], a0[:], s0[:])
                nc.vector.tensor_mul(g1[:], a1[:], s1[:])
                nc.scalar.dma_start(glu_in[0:HGO, :], g0[:])
                nc.scalar.dma_start(glu_in[HGO:GO, :], g1[:])

                nc.gpsimd.collective_compute(
                    "AllGather", OP.bypass, replica_groups=RG,
                    ins=[glu_in.opt()], outs=[glu_out.opt()])

                # ---- Phase E: readout MLP
                ps_h = pB.tile([R_HID, B], f32, tag="ph")
                gf_all = de.tile([128, MT * B], f32r, tag="gfall")
                src_g = glu_out.rearrange("(m p) b -> p m b", p=128)
                nc.scalar.dma_start(
                    gf_all[:].rearrange("p (m b) -> p m b", m=MT), src_g)
                for k in range(MT):
                    nc.tensor.matmul(ps_h[:], w1s[k],
                                     gf_all[:, k * B:(k + 1) * B],
                                     start=(k == 0), stop=(k == MT - 1))
                h1 = de.tile([R_HID, B], f32r, tag="h1")
                nc.scalar.activation(h1[:], ps_h[:], AF.Relu, bias=t_b1)
                ps_o = pB.tile([NCLS, B], f32, tag="po")
                nc.tensor.matmul(ps_o[:], w2, h1[:], start=True, stop=True)
                o_sb = de.tile([NCLS, B], f32, tag="osb")
                nc.scalar.activation(o_sb[:], ps_o[:], AF.Identity,
                                     bias=t_b2)
                nc.scalar.dma_start(d_out, o_sb[:])

    nc.compile()
    return nc


def _prep_inputs(inputs):
    x = np.asarray(inputs["x"], dtype=np.float32)
    Wb = np.asarray(inputs["Wb"], dtype=np.float32)
    bb = np.asarray(inputs["bb"], dtype=np.float32)
    log_dt = np.asarray(inputs["log_dt"], dtype=np.float32)
    C = np.asarray(inputs["C"], dtype=np.float32)
    logA = np.asarray(inputs["log_A_real"], dtype=np.float32)
    D = np.asarray(inputs["D"], dtype=np.float32)
    Wc = np.asarray(inputs["Wc"], dtype=np.float32)
    bc = np.asarray(inputs["bc"], dtype=np.float32)
    W1 = np.asarray(inputs["W1"], dtype=np.float32)
    b1 = np.asarray(inputs["b1"], dtype=np.float32)
    W2 = np.asarray(inputs["W2"], dtype=np.float32)
    b2 = np.asarray(inputs["b2"], dtype=np.float32)

    xT = np.ascontiguousarray(x.reshape(TOK, DIN).T)     # (DIN, 256)
    wT = np.ascontiguousarray(Wb.T)                      # (DIN, 1280)
    WcT = np.ascontiguousarray(Wc.T)                     # (1280, 2560)
    W1T = np.ascontiguousarray(W1.T)                     # (1280, 64)
    W2T = np.ascontiguousarray(W2.T)                     # (64, 60)
    rev = np.arange(T - 1, -1, -1, dtype=np.float32)     # 63 - t
    rev_full = np.ascontiguousarray(
        np.broadcast_to(np.repeat(rev, N2), (128, T * N2)))

    # partition-major repack: arr_p[p, k, :] = arr[k*128+p, :]
    pm = lambda a: np.ascontiguousarray(
        a.reshape(-1, 128, a.shape[-1]).transpose(1, 0, 2)).reshape(128, -1)

    packedA = np.ascontiguousarray(np.concatenate(
        [rev_full, pm(log_dt.reshape(H, 1)), pm(bb.reshape(H, 1)),
         pm(D.reshape(H, 1)), pm(logA), pm(C)], axis=1))

    def pad128(a):
        out = np.zeros((128, a.shape[1]), np.float32)
        out[:a.shape[0]] = a
        return out

    in_maps = []
    for i in range(NCORES):
        klo = i * KS
        go = i * GO
        wTp = np.ascontiguousarray(
            wT[klo:klo + KS].reshape(KT, 128, H).transpose(1, 0, 2)
        ).reshape(128, KT * H)
        xTp = np.ascontiguousarray(
            xT[klo:klo + KS].reshape(KT, 128, TOK).transpose(1, 0, 2)
        ).reshape(128, KT * TOK)
        wcT_sl = np.concatenate(
            [WcT[:, go:go + HGO], WcT[:, go + HGO:go + GO],
             WcT[:, H + go:H + go + HGO], WcT[:, H + go + HGO:H + go + GO]],
            axis=1)                                     # (1280, 320)
        packedB = np.ascontiguousarray(np.concatenate(
            [pm(wcT_sl), pm(W1T), pad128(W2T)], axis=1))
        bc_sl = np.stack(
            [bc[go:go + HGO], bc[go + HGO:go + GO],
             bc[H + go:H + go + HGO], bc[H + go + HGO:H + go + GO]],
            axis=1)                                     # (80, 4)
        packedS = np.ascontiguousarray(np.concatenate(
            [pad128(b1.reshape(R_HID, 1)), pad128(b2.reshape(NCLS, 1)),
             pad128(bc_sl)], axis=1))
        in_maps.append({
            "xT": xTp, "wT": wTp, "packedA": packedA,
            "packedB": packedB, "packedS": packedS,
        })
    return in_maps


def kernel(**inputs):
    global _compiled
    if _compiled is None:
        _compiled = _build()
    nc = _compiled
    in_maps = _prep_inputs(inputs)
    from concourse import bass_utils
    res = bass_utils.run_bass_kernel_spmd(nc, in_maps,
                                          core_ids=list(range(NCORES)))
    out = res.results[0]["out"]  # (NCLS, B)
    return np.ascontiguousarray(out.T).astype(np.float32)
